# revision 1
# baseline (speedup 1.0000x reference)
"""LocalTransformerEncoderLayer on 8 trn2 NeuronCores.

Sharding: core c = 2*b + h handles batch b, sequence half h (4096 tokens,
plus a 64-token halo on each side for the local-attention window).
Everything is done on-device per core; no collectives needed.

Layout plan (per core):
  srcT  [512, 4224] bf16  d-major haloed chunk (host-transposed)  -> QKV rhs/lhsT
  qT,kT [128,4,4224] bf16 d-major in SBUF (PE: W.T @ srcT)
  v     [128,33,512] bf16 token-major in SBUF (PE: srcT.T @ Wv)
  per q-pair p (128 query tokens, 256 keys = ext tiles p,p+1):
    simT [128keys, 2*128q] psum  = kT.T @ qT  (+ rank-1 -1e10 corner masks)
    expT bf16 = ACT exp(scale*simT);  denom = expT.T @ ones (PE);
    av [128q,512] = expT.T @ v;  out = ACT(av * (1/denom)) + src; LN1 (DVE)
    xT via PE transpose (bf16)
  FFN per 512-token block: h[f,tok] = relu(W1.T @ xT), y[tok,d] = h.T @ W2
  residual2 + LN2 token-major, DMA out fp32.
"""
import os
import numpy as np
import ml_dtypes

_BF16 = ml_dtypes.bfloat16

B, N, D, F, W = 4, 8192, 512, 2048, 64
T = N // 2            # own tokens per core = 4096
H = 64                # halo
TEXT = T + 2 * H      # 4224
NPAIR = T // 128      # 32 q-pairs per core
NBLK = T // 512       # 8 blocks
NEG = -1e10
SCALE = float(D) ** -0.5

_cache = {}


def _build(apply_bv, apply_b2, apply_ln1g, apply_ln1b, apply_ln2g, apply_ln2b):
    import concourse.bacc as bacc
    import concourse.tile as tile
    from concourse import mybir
    import concourse.bass as bass

    f32 = mybir.dt.float32
    bf16 = mybir.dt.bfloat16
    AF = mybir.ActivationFunctionType
    ALU = mybir.AluOpType

    nc = bacc.Bacc("TRN2", target_bir_lowering=False, debug=False)

    # ---- DRAM I/O ----
    srcT_d = nc.dram_tensor("srcT", [D, TEXT], bf16, kind="ExternalInput").ap()
    src_d = nc.dram_tensor("src", [T, D], f32, kind="ExternalInput").ap()
    wq_d = nc.dram_tensor("wq", [D, D], bf16, kind="ExternalInput").ap()
    wk_d = nc.dram_tensor("wk", [D, D], bf16, kind="ExternalInput").ap()
    wv_d = nc.dram_tensor("wv", [D, D], bf16, kind="ExternalInput").ap()
    bqT_d = nc.dram_tensor("bqT", [128, 4], f32, kind="ExternalInput").ap()
    bkT_d = nc.dram_tensor("bkT", [128, 4], f32, kind="ExternalInput").ap()
    w1_d = nc.dram_tensor("w1", [D, F], bf16, kind="ExternalInput").ap()
    b1T_d = nc.dram_tensor("b1T", [128, 16], f32, kind="ExternalInput").ap()
    w2_d = nc.dram_tensor("w2", [F, D], bf16, kind="ExternalInput").ap()
    ident_d = nc.dram_tensor("ident", [128, 128], bf16, kind="ExternalInput").ap()
    uA_d = nc.dram_tensor("uA", [1, 128], bf16, kind="ExternalInput").ap()
    uB_d = nc.dram_tensor("uB", [1, 128], bf16, kind="ExternalInput").ap()
    wA_d = nc.dram_tensor("wA", [1, 128], bf16, kind="ExternalInput").ap()
    wB_d = nc.dram_tensor("wB", [1, 128], bf16, kind="ExternalInput").ap()
    wA0_d = nc.dram_tensor("wA0", [1, 128], bf16, kind="ExternalInput").ap()
    wB31_d = nc.dram_tensor("wB31", [1, 128], bf16, kind="ExternalInput").ap()
    onerow_d = nc.dram_tensor("onerow", [1, 128], bf16, kind="ExternalInput").ap()
    if apply_bv:
        bvrow_d = nc.dram_tensor("bvrow", [1, D], bf16, kind="ExternalInput").ap()
    if apply_b2:
        b2row_d = nc.dram_tensor("b2row", [1, D], bf16, kind="ExternalInput").ap()
    # replicated LN params (only declared when needed)
    if apply_ln1g:
        g1_d = nc.dram_tensor("g1", [128, D], f32, kind="ExternalInput").ap()
    if apply_ln1b:
        be1_d = nc.dram_tensor("be1", [128, D], f32, kind="ExternalInput").ap()
    if apply_ln2g:
        g2_d = nc.dram_tensor("g2", [128, D], f32, kind="ExternalInput").ap()
    if apply_ln2b:
        be2_d = nc.dram_tensor("be2", [128, D], f32, kind="ExternalInput").ap()
    out_d = nc.dram_tensor("out", [T, D], f32, kind="ExternalOutput").ap()
    v_d = nc.dram_tensor("vscratch", [33 * 128, D], bf16).ap()

    from contextlib import ExitStack
    with tile.TileContext(nc) as tc, ExitStack() as ctx:
        # ---- persistent pools ----
        consts = ctx.enter_context(tc.tile_pool(name="consts", bufs=1))
        kv = ctx.enter_context(tc.tile_pool(name="kv", bufs=1))
        big_ps = ctx.enter_context(tc.tile_pool(name="big_ps", bufs=2, space="PSUM"))
        av_ps = ctx.enter_context(tc.tile_pool(name="av_ps", bufs=2, space="PSUM"))
        sim_ps = ctx.enter_context(tc.tile_pool(name="sim_ps", bufs=3, space="PSUM"))
        xt_ps = ctx.enter_context(tc.tile_pool(name="xt_ps", bufs=1, space="PSUM"))


        # constants
        wq_sb = consts.tile([128, 4, D], bf16, tag="wq")
        nc.sync.dma_start(wq_sb, wq_d.rearrange("(kt p) m -> p kt m", p=128))
        wk_sb = consts.tile([128, 4, D], bf16, tag="wk")
        nc.sync.dma_start(wk_sb, wk_d.rearrange("(kt p) m -> p kt m", p=128))
        wv_sb = consts.tile([128, 4, D], bf16, tag="wv")
        nc.sync.dma_start(wv_sb, wv_d.rearrange("(kt p) m -> p kt m", p=128))
        w1_sb = consts.tile([128, 4, F], bf16, tag="w1")
        w2_sb = consts.tile([128, 16, D], bf16, tag="w2")
        bqT_sb = consts.tile([128, 4], f32, tag="bqT")
        nc.sync.dma_start(bqT_sb, bqT_d)
        bkT_sb = consts.tile([128, 4], f32, tag="bkT")
        nc.sync.dma_start(bkT_sb, bkT_d)
        b1T_sb = consts.tile([128, 16], f32, tag="b1T")
        nc.sync.dma_start(b1T_sb, b1T_d)
        ident_sb = consts.tile([128, 128], bf16, tag="ident")
        nc.sync.dma_start(ident_sb, ident_d)
        uA_sb = consts.tile([1, 128], bf16, tag="uA")
        nc.sync.dma_start(uA_sb, uA_d)
        uB_sb = consts.tile([1, 128], bf16, tag="uB")
        nc.sync.dma_start(uB_sb, uB_d)
        wA_sb = consts.tile([1, 128], bf16, tag="wA")
        nc.sync.dma_start(wA_sb, wA_d)
        wB_sb = consts.tile([1, 128], bf16, tag="wB")
        nc.sync.dma_start(wB_sb, wB_d)
        wA0_sb = consts.tile([1, 128], bf16, tag="wA0")
        nc.sync.dma_start(wA0_sb, wA0_d)
        wB31_sb = consts.tile([1, 128], bf16, tag="wB31")
        nc.sync.dma_start(wB31_sb, wB31_d)
        ones_sb = consts.tile([128, 1], bf16, tag="ones")
        nc.vector.memset(ones_sb, 1.0)
        eps_sb = consts.tile([128, 1], f32, tag="eps")
        nc.vector.memset(eps_sb, 1e-5)
        if apply_bv:
            onerow_sb = consts.tile([1, 128], bf16, tag="onerow")
            nc.sync.dma_start(onerow_sb, onerow_d)
            bvrow_sb = consts.tile([1, D], bf16, tag="bvrow")
            nc.sync.dma_start(bvrow_sb, bvrow_d)
        if apply_b2:
            onerow2_sb = consts.tile([1, 128], bf16, tag="onerow2")
            nc.sync.dma_start(onerow2_sb, onerow_d)
            b2row_sb = consts.tile([1, D], bf16, tag="b2row")
            nc.sync.dma_start(b2row_sb, b2row_d)
        if apply_ln1g:
            g1_sb = consts.tile([128, D], f32, tag="g1")
            nc.sync.dma_start(g1_sb, g1_d)
        if apply_ln1b:
            be1_sb = consts.tile([128, D], f32, tag="be1")
            nc.sync.dma_start(be1_sb, be1_d)
        if apply_ln2g:
            g2_sb = consts.tile([128, D], f32, tag="g2")
            nc.sync.dma_start(g2_sb, g2_d)
        if apply_ln2b:
            be2_sb = consts.tile([128, D], f32, tag="be2")
            nc.sync.dma_start(be2_sb, be2_d)

        # persistent activations
        qT_sb = kv.tile([128, 4, TEXT], bf16, tag="qT")
        kT_sb = kv.tile([128, 4, TEXT], bf16, tag="kT")

        # ---- phase 1: QKV over ext grid (srcT streamed per block) ----
        srcs = ctx.enter_context(tc.tile_pool(name="srcs", bufs=3))
        kv_io = ctx.enter_context(tc.tile_pool(name="kv_io", bufs=3))
        srcT_r = srcT_d.rearrange("(dt p) t -> p dt t", p=128)
        if True:
            blocks = [(i * 512, 512) for i in range(TEXT // 512)] + [(4096, 128)]
            for off, tw in blocks:
                srcT_sb = srcs.tile([128, 4, 512], bf16, tag="srcT")
                nc.sync.dma_start(srcT_sb[:, :, :tw], srcT_r[:, :, off:off + tw])
                # qT, kT (d-major)
                for w_sb, b_sb, dst in ((wq_sb, bqT_sb, qT_sb), (wk_sb, bkT_sb, kT_sb)):
                    for dq in range(4):
                        ps = big_ps.tile([128, 512], f32, tag="big")
                        for kt in range(4):
                            nc.tensor.matmul(
                                ps[:, :tw],
                                lhsT=w_sb[:, kt, dq * 128:(dq + 1) * 128],
                                rhs=srcT_sb[:, kt, :tw],
                                start=(kt == 0), stop=(kt == 3),
                            )
                        nc.scalar.activation(
                            dst[:, dq, off:off + tw], ps[:, :tw],
                            AF.Identity, bias=b_sb[:, dq:dq + 1],
                        )
                # v (token-major), per 128-token tile
                for s in range(tw // 128):
                    ti = (off + s * 128) // 128
                    ps = big_ps.tile([128, 512], f32, tag="big")
                    for kt in range(4):
                        nc.tensor.matmul(
                            ps,
                            lhsT=srcT_sb[:, kt, s * 128:s * 128 + 128],
                            rhs=wv_sb[:, kt, :],
                            start=(kt == 0), stop=(kt == 3 and not apply_bv),
                        )
                    if apply_bv:
                        nc.tensor.matmul(ps, lhsT=onerow_sb, rhs=bvrow_sb,
                                         start=False, stop=True)
                    v_t = kv_io.tile([128, D], bf16, tag="vout")
                    nc.vector.tensor_copy(v_t, ps)
                    nc.sync.dma_start(v_d[ti * 128:(ti + 1) * 128, :], v_t)

        # FFN weights only needed ~150us in; emit their DMAs after phase 1
        nc.sync.dma_start(w1_sb, w1_d.rearrange("(kt p) m -> p kt m", p=128))
        nc.sync.dma_start(w2_sb, w2_d.rearrange("(ft p) m -> p ft m", p=128))

        # ---- phase 2: attention + FFN per 512-token block ----
        x_pool = ctx.enter_context(tc.tile_pool(name="x_pool", bufs=5))
        xT_pool = ctx.enter_context(tc.tile_pool(name="xT_pool", bufs=2))
        h_pool = ctx.enter_context(tc.tile_pool(name="h_pool", bufs=1))
        attn_pool = ctx.enter_context(tc.tile_pool(name="attn_pool", bufs=2))
        io_pool = ctx.enter_context(tc.tile_pool(name="io_pool", bufs=2))
        stat_pool = ctx.enter_context(tc.tile_pool(name="stat_pool", bufs=8))
        vpool = ctx.enter_context(tc.tile_pool(name="vpool", bufs=3))
        exp_pool = ctx.enter_context(tc.tile_pool(name="exp_pool", bufs=4))
        res_pool = ctx.enter_context(tc.tile_pool(name="res_pool", bufs=4))

        def ln_stats(s_sb, tag):
            st6 = stat_pool.tile([128, 6], f32, tag="st6")
            nc.vector.bn_stats(st6, s_sb)
            mv = stat_pool.tile([128, 2], f32, tag=tag)
            nc.vector.bn_aggr(mv, st6)
            return mv

        def ln_batch_rstd(mvs, tag):
            """batched sqrt+reciprocal over len(mvs) row-stats -> rstd [128, n]"""
            n = len(mvs)
            var_blk = stat_pool.tile([128, n], f32, tag=tag + "v")
            for j, mv in enumerate(mvs):
                nc.gpsimd.tensor_copy(var_blk[:, j:j + 1], mv[:, 1:2])
            std_blk = stat_pool.tile([128, n], f32, tag=tag + "s")
            nc.scalar.activation(std_blk, var_blk, AF.Sqrt, bias=eps_sb)
            rstd_blk = stat_pool.tile([128, n], f32, tag=tag + "r")
            nc.vector.reciprocal(rstd_blk, std_blk)
            return rstd_blk

        def ln_norm(dst, s_sb, mv, rstd_col, gamma, beta, dst2=None):
            nc.vector.tensor_scalar(dst, s_sb, mv[:, 0:1], rstd_col,
                                    ALU.subtract, ALU.mult)
            if gamma is not None:
                nc.vector.tensor_mul(dst, dst, gamma)
            if beta is not None:
                nc.vector.tensor_add(dst, dst, beta)
            if dst2 is not None:
                nc.scalar.copy(dst2, dst)

        # ---- phase 2 pipeline: sim(p) | den/av(p-1) | LN+FFN(block, lag 2) ----
        expT_t = {}
        vav = {}
        h_blks = {}
        xbf_blks = {}
        s_tiles = {}
        mv1 = {}
        x_tiles = {}
        xT_blks = {}

        def emit_sim(p):
            qoff = H + p * 128
            vA = vpool.tile([128, D], bf16, tag="vin")
            nc.sync.dma_start(vA, v_d[p * 128:(p + 1) * 128, :])
            vB = vpool.tile([128, D], bf16, tag="vin2")
            nc.sync.dma_start(vB, v_d[(p + 1) * 128:(p + 2) * 128, :])
            vav[p] = (vA, vB)
            ps_sim = sim_ps.tile([128, 256], f32, tag="sim")
            for half, (ktile, u_sb, w_vec) in enumerate((
                (p, uA_sb, wA0_sb if p == 0 else wA_sb),
                (p + 1, uB_sb, wB31_sb if p == NPAIR - 1 else wB_sb),
            )):
                reg = ps_sim[:, half * 128:(half + 1) * 128]
                for kt in range(4):
                    nc.tensor.matmul(
                        reg,
                        lhsT=kT_sb[:, kt, ktile * 128:(ktile + 1) * 128],
                        rhs=qT_sb[:, kt, qoff:qoff + 128],
                        start=(kt == 0), stop=False,
                    )
                nc.tensor.matmul(reg, lhsT=u_sb, rhs=w_vec, start=False, stop=True)
            expT = exp_pool.tile([128, 256], bf16, tag="expT")
            nc.scalar.activation(expT, ps_sim, AF.Exp, scale=SCALE)
            expT_t[p] = expT

        def emit_av(p):
            expT = expT_t.pop(p)
            vA, vB = vav.pop(p)
            ps_den = sim_ps.tile([128, 1], f32, tag="sim")
            nc.tensor.matmul(ps_den, lhsT=expT[:, 0:128], rhs=ones_sb,
                             start=True, stop=False)
            nc.tensor.matmul(ps_den, lhsT=expT[:, 128:256], rhs=ones_sb,
                             start=False, stop=True)
            recip = stat_pool.tile([128, 1], f32, tag="recip")
            nc.vector.reciprocal(recip, ps_den)
            ps_av = av_ps.tile([128, 512], f32, tag="av")
            nc.tensor.matmul(ps_av, lhsT=expT[:, 0:128], rhs=vA,
                             start=True, stop=False)
            nc.tensor.matmul(ps_av, lhsT=expT[:, 128:256], rhs=vB,
                             start=False, stop=True)
            t_sb = attn_pool.tile([128, D], f32, tag="t")
            nc.vector.tensor_scalar(t_sb, ps_av, recip, None, ALU.mult)
            src_t = io_pool.tile([128, D], f32, tag="srct")
            nc.sync.dma_start(src_t, src_d[p * 128:(p + 1) * 128, :])
            s_sb = res_pool.tile([128, D], f32, tag="s")
            nc.vector.tensor_add(s_sb, t_sb, src_t)
            s_tiles[p] = s_sb
            mv1[p] = ln_stats(s_sb, "mv1")

        def ln_compute(blk):
            mvs = [mv1.pop(blk * 4 + j) for j in range(4)]
            rstd1 = ln_batch_rstd(mvs, "b1")
            xbfs = []
            for j in range(4):
                p = blk * 4 + j
                x_f32 = x_pool.tile([128, D], f32, tag="xf32")
                x_bf = attn_pool.tile([128, D], bf16, tag="xbf")
                ln_norm(x_f32, s_tiles.pop(p), mvs[j], rstd1[:, j:j + 1],
                        g1_sb if apply_ln1g else None,
                        be1_sb if apply_ln1b else None, dst2=x_bf)
                x_tiles[p] = x_f32
                xbfs.append(x_bf)
            xbf_blks[blk] = xbfs

        def emit_transposes(blk):
            xT_blk = xT_pool.tile([128, 4, 512], bf16, tag="xT")
            xT_blks[blk] = xT_blk
            xbfs = xbf_blks.pop(blk)
            for j in range(4):
                for dt in range(4):
                    ps_xt = xt_ps.tile([128, 128], bf16, tag="xt")
                    nc.tensor.transpose(ps_xt, xbfs[j][:, dt * 128:(dt + 1) * 128],
                                        ident_sb)
                    nc.vector.tensor_copy(xT_blk[:, dt, j * 128:(j + 1) * 128], ps_xt)

        def emit_ffn_h(blk):
            xT_blk = xT_blks.pop(blk)
            h_sb = h_pool.tile([128, 16, 512], bf16, tag="h")
            for ft in range(16):
                ps_h = big_ps.tile([128, 512], f32, tag="big")
                for kt in range(4):
                    nc.tensor.matmul(
                        ps_h,
                        lhsT=w1_sb[:, kt, ft * 128:(ft + 1) * 128],
                        rhs=xT_blk[:, kt, :],
                        start=(kt == 0), stop=(kt == 3),
                    )
                nc.scalar.activation(h_sb[:, ft, :], ps_h, AF.Relu,
                                     bias=b1T_sb[:, ft:ft + 1])
            h_blks[blk] = h_sb

        def emit_ffn_y(blk):
            h_sb = h_blks.pop(blk)
            s2_tiles = []
            mv2 = []
            for j in range(4):
                p = blk * 4 + j
                ps_y = big_ps.tile([128, 512], f32, tag="big")
                for ft in range(16):
                    nc.tensor.matmul(
                        ps_y,
                        lhsT=h_sb[:, ft, j * 128:(j + 1) * 128],
                        rhs=w2_sb[:, ft, :],
                        start=(ft == 0), stop=(ft == 15 and not apply_b2),
                    )
                if apply_b2:
                    nc.tensor.matmul(ps_y, lhsT=onerow2_sb, rhs=b2row_sb,
                                     start=False, stop=True)
                s2 = res_pool.tile([128, D], f32, tag="s2")
                nc.vector.tensor_add(s2, x_tiles.pop(p), ps_y)
                s2_tiles.append(s2)
                mv2.append(ln_stats(s2, "mv2"))
            rstd2 = ln_batch_rstd(mv2, "b2")
            for j in range(4):
                p = blk * 4 + j
                o_sb = io_pool.tile([128, D], f32, tag="o")
                ln_norm(o_sb, s2_tiles[j], mv2[j], rstd2[:, j:j + 1],
                        g2_sb if apply_ln2g else None,
                        be2_sb if apply_ln2b else None)
                nc.sync.dma_start(out_d[p * 128:(p + 1) * 128, :], o_sb)

        xbf_blks = {}
        h_blks = {}
        for p in range(NPAIR + 4):
            if p < NPAIR:
                emit_sim(p)
            if 1 <= p <= NPAIR:
                emit_av(p - 1)
            if p >= 5 and (p - 5) % 4 == 0:
                ln_compute((p - 5) // 4)
            if p >= 6 and (p - 6) % 4 == 0:
                emit_transposes((p - 6) // 4)
                emit_ffn_h((p - 6) // 4)
            if p >= 7 and (p - 7) % 4 == 0:
                emit_ffn_y((p - 7) // 4)

    nc.compile()
    return nc


def _get_program(key):
    if key not in _cache:
        _cache[key] = _build(*key)
    return _cache[key]


last_exec_ns = None


def _install_ntff_hook():
    """NTFF profiling hook for axon (normally installed via antenv.axon_hooks)."""
    import sys, types
    if 'antenv.axon_hooks' in sys.modules:
        return
    mod = types.ModuleType('antenv.axon_hooks')
    _h = [None]
    mod.set_axon_ntff_profile_hook = lambda h: _h.__setitem__(0, h)
    mod.get_axon_ntff_profile_hook = lambda: _h[0]
    sys.modules['antenv.axon_hooks'] = mod
    import antenv
    antenv.axon_hooks = mod
    try:
        from trn_agent_boot.trn_boot import _ntff_profile_via_ctypes
        mod.set_axon_ntff_profile_hook(
            _ntff_profile_via_ctypes('/opt/axon/libaxon_pjrt.so'))
    except Exception:
        pass


def kernel(src, mask, Wq, bq, Wk, bk, Wv, bv, ln1_g, ln1_b,
           W1, b1, W2, b2, ln2_g, ln2_b):
    global last_exec_ns
    src = np.asarray(src, np.float32)
    if not bool(np.asarray(mask).all()):
        raise NotImplementedError("only all-true mask supported")

    key = (bool(np.any(bv)), bool(np.any(b2)),
           not bool(np.all(ln1_g == 1)), bool(np.any(ln1_b)),
           not bool(np.all(ln2_g == 1)), bool(np.any(ln2_b)))
    nc = _get_program(key)
    apply_bv, apply_b2, a_g1, a_b1, a_g2, a_b2 = key

    qi = np.arange(128)
    wA = np.where(qi >= 64, NEG, 0.0).astype(_BF16).reshape(1, 128)
    wB = np.where(qi < 64, NEG, 0.0).astype(_BF16).reshape(1, 128)
    wfull = np.full((1, 128), NEG, _BF16)
    uA = (qi < 64).astype(_BF16).reshape(1, 128)
    uB = (qi >= 64).astype(_BF16).reshape(1, 128)

    shared = {
        "wq": Wq.astype(_BF16), "wk": Wk.astype(_BF16), "wv": Wv.astype(_BF16),
        "bqT": np.asarray(bq, np.float32).reshape(4, 128).T.copy(),
        "bkT": np.asarray(bk, np.float32).reshape(4, 128).T.copy(),
        "w1": W1.astype(_BF16),
        "b1T": np.asarray(b1, np.float32).reshape(16, 128).T.copy(),
        "w2": W2.astype(_BF16),
        "ident": np.eye(128, dtype=_BF16),
        "uA": uA, "uB": uB, "wA": wA, "wB": wB,
        "onerow": np.ones((1, 128), _BF16),
    }
    if apply_bv:
        shared["bvrow"] = np.asarray(bv, np.float32).reshape(1, D).astype(_BF16)
    if apply_b2:
        shared["b2row"] = np.asarray(b2, np.float32).reshape(1, D).astype(_BF16)
    if a_g1:
        shared["g1"] = np.tile(np.asarray(ln1_g, np.float32).reshape(1, D), (128, 1))
    if a_b1:
        shared["be1"] = np.tile(np.asarray(ln1_b, np.float32).reshape(1, D), (128, 1))
    if a_g2:
        shared["g2"] = np.tile(np.asarray(ln2_g, np.float32).reshape(1, D), (128, 1))
    if a_b2:
        shared["be2"] = np.tile(np.asarray(ln2_b, np.float32).reshape(1, D), (128, 1))

    in_maps = []
    for c in range(8):
        b, h = divmod(c, 2)
        start = h * T - H
        ext = np.zeros((TEXT, D), np.float32)
        lo, hi = max(start, 0), min(start + TEXT, N)
        ext[lo - start: hi - start] = src[b, lo:hi]
        m = dict(shared)
        m["srcT"] = np.ascontiguousarray(ext.T).astype(_BF16)
        m["src"] = np.ascontiguousarray(src[b, h * T:(h + 1) * T])
        m["wA0"] = wfull if h == 0 else wA
        m["wB31"] = wfull if h == 1 else wB
        in_maps.append(m)

    from concourse.bass_utils import run_bass_kernel_spmd
    trace = bool(os.environ.get("KERNEL_TRACE"))
    if trace:
        _install_ntff_hook()
    res = run_bass_kernel_spmd(nc, in_maps, core_ids=list(range(8)), trace=trace)
    if trace:
        last_exec_ns = res.exec_time_ns

    out = np.empty((B, N, D), np.float32)
    for c in range(8):
        b, h = divmod(c, 2)
        out[b, h * T:(h + 1) * T] = res.results[c]["out"]
    return out



# revision 16
# speedup vs baseline: 1.1612x; 1.1612x over previous
"""LocalTransformerEncoderLayer on 8 trn2 NeuronCores.

Sharding: core c = 2*b + h handles batch b, sequence half h (4096 tokens,
plus a 64-token halo on each side for the local-attention window).
Everything is done on-device per core; no collectives needed.

v2 schedule: per-pair software pipeline so the PE never stalls on the
DVE FIFO (the v1 per-block LN chains caused ~9us PE idle per block and
HAM re-throttling).

Layout plan (per core):
  srcT  [512, 4224] bf16  d-major haloed chunk (host-transposed)  -> QKV rhs/lhsT
  qT,kT [128,4,4224] bf16 d-major in SBUF (PE: W.T @ srcT)
  v     [128,33,512] bf16 token-major in SBUF (PE: srcT.T @ Wv)
  per q-pair p (128 query tokens, 256 keys = ext tiles p,p+1):
    step p:   simT [256 keys, 128 q] psum (+rank-1 corner masks); exp (ACT);
              den = expT.T @ ones (PE, same psum bank)
    step p+1: av = expT.T @ v (PE); s = av*recip + src (DVE stt, accum=sum);
              sumsq via ACT square(accum); m/var (gpsimd); sqrt (ACT);
              rstd (DVE); x_bf = (s-m)*rstd (gpsimd, bf16)
    step p+2: 4 PE transposes of x_bf into one psum bank; 1 strided copy out
  FFN per 512-token block b (steps 4b+6 / 4b+7):
    h[f,tok] = relu(W1.T @ xT) (PE + ACT/DVE split relu)
    y[tok,d] = h.T @ W2 (PE); s2 = y + x (DVE stt, accum); LN2 stats like LN1
  step 4b+8: o = (s2-m2)*rstd2 (gpsimd); DMA out fp32.
"""
import os
import numpy as np
import ml_dtypes

_BF16 = ml_dtypes.bfloat16

B, N, D, F, W = 4, 8192, 512, 2048, 64
T = N // 2            # own tokens per core = 4096
H = 64                # halo
TEXT = T + 2 * H      # 4224
NPAIR = T // 128      # 32 q-pairs per core
NBLK = T // 512       # 8 blocks
NEG = -1e10
SCALE = float(D) ** -0.5
INV_D = 1.0 / float(D)

_cache = {}


def _build(apply_bv, apply_b2, apply_ln1g, apply_ln1b, apply_ln2g, apply_ln2b):
    import concourse.bacc as bacc
    import concourse.tile as tile
    from concourse import mybir
    import concourse.bass as bass

    f32 = mybir.dt.float32
    bf16 = mybir.dt.bfloat16
    AF = mybir.ActivationFunctionType
    ALU = mybir.AluOpType

    nc = bacc.Bacc("TRN2", target_bir_lowering=False, debug=False)

    # ---- DRAM I/O ----
    srcT_d = nc.dram_tensor("srcT", [D, TEXT], bf16, kind="ExternalInput").ap()
    srcbf_d = nc.dram_tensor("srcbf", [T, D], bf16, kind="ExternalInput").ap()
    wq_d = nc.dram_tensor("wq", [D, D], bf16, kind="ExternalInput").ap()
    wk_d = nc.dram_tensor("wk", [D, D], bf16, kind="ExternalInput").ap()
    wv_d = nc.dram_tensor("wv", [D, D], bf16, kind="ExternalInput").ap()
    bqT_d = nc.dram_tensor("bqT", [128, 4], f32, kind="ExternalInput").ap()
    bkT_d = nc.dram_tensor("bkT", [128, 4], f32, kind="ExternalInput").ap()
    w1_d = nc.dram_tensor("w1", [D, F], bf16, kind="ExternalInput").ap()
    b1T_d = nc.dram_tensor("b1T", [128, 16], f32, kind="ExternalInput").ap()
    w2_d = nc.dram_tensor("w2", [F, D], bf16, kind="ExternalInput").ap()
    ident_d = nc.dram_tensor("ident", [128, 128], bf16, kind="ExternalInput").ap()
    uA_d = nc.dram_tensor("uA", [1, 128], bf16, kind="ExternalInput").ap()
    uB_d = nc.dram_tensor("uB", [1, 128], bf16, kind="ExternalInput").ap()
    wA_d = nc.dram_tensor("wA", [1, 128], bf16, kind="ExternalInput").ap()
    wB_d = nc.dram_tensor("wB", [1, 128], bf16, kind="ExternalInput").ap()
    wA0_d = nc.dram_tensor("wA0", [1, 128], bf16, kind="ExternalInput").ap()
    wB31_d = nc.dram_tensor("wB31", [1, 128], bf16, kind="ExternalInput").ap()
    onerow_d = nc.dram_tensor("onerow", [1, 128], bf16, kind="ExternalInput").ap()
    if apply_bv:
        bvrow_d = nc.dram_tensor("bvrow", [1, D], bf16, kind="ExternalInput").ap()
    if apply_b2:
        b2row_d = nc.dram_tensor("b2row", [1, D], bf16, kind="ExternalInput").ap()
    if apply_ln1g:
        g1_d = nc.dram_tensor("g1", [128, D], f32, kind="ExternalInput").ap()
    if apply_ln1b:
        be1_d = nc.dram_tensor("be1", [128, D], f32, kind="ExternalInput").ap()
    if apply_ln2g:
        g2_d = nc.dram_tensor("g2", [128, D], f32, kind="ExternalInput").ap()
    if apply_ln2b:
        be2_d = nc.dram_tensor("be2", [128, D], f32, kind="ExternalInput").ap()
    out_d = nc.dram_tensor("out", [T, D], f32, kind="ExternalOutput").ap()

    from contextlib import ExitStack
    with tile.TileContext(nc) as tc, ExitStack() as ctx:
        # ---- pools ----
        consts = ctx.enter_context(tc.tile_pool(name="consts", bufs=1))
        kv = ctx.enter_context(tc.tile_pool(name="kv", bufs=1))
        big_ps = ctx.enter_context(tc.tile_pool(name="big_ps", bufs=2, space="PSUM"))
        av_ps = ctx.enter_context(tc.tile_pool(name="av_ps", bufs=2, space="PSUM"))
        sim_ps = ctx.enter_context(tc.tile_pool(name="sim_ps", bufs=2, space="PSUM"))
        xt_ps = ctx.enter_context(tc.tile_pool(name="xt_ps", bufs=2, space="PSUM"))

        srcs = ctx.enter_context(tc.tile_pool(name="srcs", bufs=2))
        io_pool = ctx.enter_context(tc.tile_pool(name="io_pool", bufs=2))
        exp_pool = ctx.enter_context(tc.tile_pool(name="exp_pool", bufs=2))
        s_pool = ctx.enter_context(tc.tile_pool(name="s_pool", bufs=2))
        xbf_pool = ctx.enter_context(tc.tile_pool(name="xbf_pool", bufs=7))
        xT_pool = ctx.enter_context(tc.tile_pool(name="xT_pool", bufs=2))
        h_pool = ctx.enter_context(tc.tile_pool(name="h_pool", bufs=1))
        s2_pool = ctx.enter_context(tc.tile_pool(name="s2_pool", bufs=4))
        o_pool = ctx.enter_context(tc.tile_pool(name="o_pool", bufs=3))
        stat_pool = ctx.enter_context(tc.tile_pool(name="stat_pool", bufs=4))
        junk_pool = ctx.enter_context(tc.tile_pool(name="junk_pool", bufs=1))

        # ---- load-bearing DMAs first: srcT chunk 0, wq, wk ----
        srcT_r = srcT_d.rearrange("(dt p) t -> p dt t", p=128)
        blocks = [(i * 512, 512) for i in range(TEXT // 512)] + [(4096, 128)]
        srcT_tiles = {}
        off0, tw0 = blocks[0]
        srcT0 = srcs.tile([128, 4, 512], bf16, tag="srcT")
        nc.sync.dma_start(srcT0[:, :, :tw0], srcT_r[:, :, off0:off0 + tw0])
        srcT_tiles[0] = srcT0
        wq_sb = consts.tile([128, 4, D], bf16, tag="wq")
        nc.sync.dma_start(wq_sb, wq_d.rearrange("(kt p) m -> p kt m", p=128))
        wk_sb = consts.tile([128, 4, D], bf16, tag="wk")
        nc.sync.dma_start(wk_sb, wk_d.rearrange("(kt p) m -> p kt m", p=128))
        bqT_sb = consts.tile([128, 4], f32, tag="bqT")
        nc.sync.dma_start(bqT_sb, bqT_d)
        bkT_sb = consts.tile([128, 4], f32, tag="bkT")
        nc.sync.dma_start(bkT_sb, bkT_d)
        wv_sb = consts.tile([128, 4, D], bf16, tag="wv")
        nc.sync.dma_start(wv_sb, wv_d.rearrange("(kt p) m -> p kt m", p=128))

        w1_sb = consts.tile([128, 4, F], bf16, tag="w1")
        w2_sb = consts.tile([128, 16, D], bf16, tag="w2")
        b1T_sb = consts.tile([128, 16], f32, tag="b1T")
        nc.sync.dma_start(b1T_sb, b1T_d)
        ident_sb = consts.tile([128, 128], bf16, tag="ident")
        nc.sync.dma_start(ident_sb, ident_d)
        uA_sb = consts.tile([1, 128], bf16, tag="uA")
        nc.sync.dma_start(uA_sb, uA_d)
        uB_sb = consts.tile([1, 128], bf16, tag="uB")
        nc.sync.dma_start(uB_sb, uB_d)
        wA_sb = consts.tile([1, 128], bf16, tag="wA")
        nc.sync.dma_start(wA_sb, wA_d)
        wB_sb = consts.tile([1, 128], bf16, tag="wB")
        nc.sync.dma_start(wB_sb, wB_d)
        wA0_sb = consts.tile([1, 128], bf16, tag="wA0")
        nc.sync.dma_start(wA0_sb, wA0_d)
        wB31_sb = consts.tile([1, 128], bf16, tag="wB31")
        nc.sync.dma_start(wB31_sb, wB31_d)
        ones_sb = consts.tile([128, 1], bf16, tag="ones")
        nc.vector.memset(ones_sb, 1.0)
        eps_sb = consts.tile([128, 1], f32, tag="eps")
        nc.vector.memset(eps_sb, 1e-5)
        if apply_bv:
            onerow_sb = consts.tile([1, 128], bf16, tag="onerow")
            nc.sync.dma_start(onerow_sb, onerow_d)
            bvrow_sb = consts.tile([1, D], bf16, tag="bvrow")
            nc.sync.dma_start(bvrow_sb, bvrow_d)
        if apply_b2:
            onerow2_sb = consts.tile([1, 128], bf16, tag="onerow2")
            nc.sync.dma_start(onerow2_sb, onerow_d)
            b2row_sb = consts.tile([1, D], bf16, tag="b2row")
            nc.sync.dma_start(b2row_sb, b2row_d)
        if apply_ln1g:
            g1_sb = consts.tile([128, D], f32, tag="g1")
            nc.sync.dma_start(g1_sb, g1_d)
        if apply_ln1b:
            be1_sb = consts.tile([128, D], f32, tag="be1")
            nc.sync.dma_start(be1_sb, be1_d)
        if apply_ln2g:
            g2_sb = consts.tile([128, D], f32, tag="g2")
            nc.sync.dma_start(g2_sb, g2_d)
        if apply_ln2b:
            be2_sb = consts.tile([128, D], f32, tag="be2")
            nc.sync.dma_start(be2_sb, be2_d)

        # persistent activations (qT only covers own tokens, no halo)
        qT_sb = kv.tile([128, 4, T], bf16, tag="qT")
        kT_sb = kv.tile([128, 4, TEXT], bf16, tag="kT")
        v_sb = kv.tile([128, 33, D], bf16, tag="v")

        # ---- phase 1: QKV over ext grid ----
        for bi, (off, tw) in enumerate(blocks):
            if bi + 1 < len(blocks):
                noff, ntw = blocks[bi + 1]
                srcT_n = srcs.tile([128, 4, 512], bf16, tag="srcT")
                nc.sync.dma_start(srcT_n[:, :, :ntw], srcT_r[:, :, noff:noff + ntw])
                srcT_tiles[bi + 1] = srcT_n
            srcT_sb = srcT_tiles.pop(bi)
            # q range clipped to own tokens [H, H+T) in ext coords
            qlo, qhi = max(off, H), min(off + tw, H + T)
            # qT, kT (d-major)
            for w_sb, b_sb, dst, lo, hi, doff in (
                (wq_sb, bqT_sb, qT_sb, qlo, qhi, -H),
                (wk_sb, bkT_sb, kT_sb, off, off + tw, 0),
            ):
                if lo >= hi:
                    continue
                for dq in range(4):
                    ps = big_ps.tile([128, 512], f32, tag="big")
                    for kt in range(4):
                        nc.tensor.matmul(
                            ps[:, :tw],
                            lhsT=w_sb[:, kt, dq * 128:(dq + 1) * 128],
                            rhs=srcT_sb[:, kt, :tw],
                            start=(kt == 0), stop=(kt == 3),
                        )
                    nc.scalar.activation(
                        dst[:, dq, lo + doff:hi + doff],
                        ps[:, lo - off:hi - off],
                        AF.Identity, bias=b_sb[:, dq:dq + 1],
                    )
            # v (token-major) into SBUF, per 128-token tile
            for s in range(tw // 128):
                ti = (off + s * 128) // 128
                ps = big_ps.tile([128, 512], f32, tag="big")
                for kt in range(4):
                    nc.tensor.matmul(
                        ps,
                        lhsT=srcT_sb[:, kt, s * 128:s * 128 + 128],
                        rhs=wv_sb[:, kt, :],
                        start=(kt == 0), stop=(kt == 3 and not apply_bv),
                    )
                if apply_bv:
                    nc.tensor.matmul(ps, lhsT=onerow_sb, rhs=bvrow_sb,
                                     start=False, stop=True)
                nc.vector.tensor_copy(v_sb[:, ti, :], ps)

        # FFN weights needed ~150us in; emit their DMAs after phase 1
        nc.sync.dma_start(w1_sb, w1_d.rearrange("(kt p) m -> p kt m", p=128))
        nc.sync.dma_start(w2_sb, w2_d.rearrange("(ft p) m -> p ft m", p=128))

        # ---- phase 2 state ----
        simden = {}       # p -> psum tile: [:, 0:256] simT, [:, 256:257] den
        expT_t = {}
        src_t = {}        # residual src tiles (bf16)
        s_tiles = {}      # p -> (s_f32, m, rstd)
        xbf = {}          # p -> bf16 LN1 output
        xT_blks = {}
        h_blks = {}
        ln2 = {}          # b -> list of (s2, m2, rstd2)

        def emit_src_prefetch(p):
            t = io_pool.tile([128, D], bf16, tag="srct")
            nc.sync.dma_start(t, srcbf_d[p * 128:(p + 1) * 128, :])
            src_t[p] = t

        def emit_sim(p):
            qoff = p * 128
            ps = sim_ps.tile([128, 512], f32, tag="sim")
            simden[p] = ps
            for half, (ktile, u_sb, w_vec) in enumerate((
                (p, uA_sb, wA0_sb if p == 0 else wA_sb),
                (p + 1, uB_sb, wB31_sb if p == NPAIR - 1 else wB_sb),
            )):
                reg = ps[:, half * 128:(half + 1) * 128]
                for kt in range(4):
                    nc.tensor.matmul(
                        reg,
                        lhsT=kT_sb[:, kt, ktile * 128:(ktile + 1) * 128],
                        rhs=qT_sb[:, kt, qoff:qoff + 128],
                        start=(kt == 0), stop=False,
                    )
                nc.tensor.matmul(reg, lhsT=u_sb, rhs=w_vec, start=False, stop=True)
            expT = exp_pool.tile([128, 256], bf16, tag="expT")
            nc.scalar.activation(expT, ps[:, 0:256], AF.Exp, scale=SCALE)
            expT_t[p] = expT

        def emit_av_mms(p):
            # den + av matmuls; all inputs were produced during step p, so
            # these never make the PE wait on another engine.
            expT = expT_t.pop(p)
            ps_sd = simden[p]
            nc.tensor.matmul(ps_sd[:, 256:257], lhsT=expT[:, 0:128], rhs=ones_sb,
                             start=True, stop=False)
            nc.tensor.matmul(ps_sd[:, 256:257], lhsT=expT[:, 128:256], rhs=ones_sb,
                             start=False, stop=True)
            ps_av = av_ps.tile([128, 512], f32, tag="av")
            nc.tensor.matmul(ps_av, lhsT=expT[:, 0:128], rhs=v_sb[:, p, :],
                             start=True, stop=False)
            nc.tensor.matmul(ps_av, lhsT=expT[:, 128:256], rhs=v_sb[:, p + 1, :],
                             start=False, stop=True)
            return ps_av

        def emit_pairchain(p, ps_av):
            ps_sd = simden.pop(p)
            recip = stat_pool.tile([128, 1], f32, tag="recip", bufs=2)
            nc.vector.reciprocal(recip, ps_sd[:, 256:257])
            s_sb = s_pool.tile([128, D], f32, tag="s")
            ssum = stat_pool.tile([128, 1], f32, tag="ssum", bufs=2)
            nc.vector.scalar_tensor_tensor(
                s_sb, ps_av, recip, src_t.pop(p),
                ALU.mult, ALU.add, accum_out=ssum)
            junk = junk_pool.tile([128, D], bf16, tag="junk")
            sumsq = stat_pool.tile([128, 1], f32, tag="sumsq", bufs=2)
            nc.scalar.activation(junk, s_sb, AF.Square, accum_out=sumsq)
            m = stat_pool.tile([128, 1], f32, tag="m", bufs=2)
            nc.vector.tensor_scalar_mul(m, ssum, INV_D)
            msq = stat_pool.tile([128, 1], f32, tag="msq", bufs=2)
            nc.vector.tensor_scalar_mul(msq, m, m)
            var = stat_pool.tile([128, 1], f32, tag="var", bufs=2)
            nc.vector.scalar_tensor_tensor(var, sumsq, INV_D, msq,
                                           ALU.mult, ALU.subtract)
            std = stat_pool.tile([128, 1], f32, tag="std", bufs=2)
            nc.scalar.activation(std, var, AF.Sqrt, bias=eps_sb)
            rstd = stat_pool.tile([128, 1], f32, tag="rstd", bufs=2)
            nc.vector.reciprocal(rstd, std)
            nmr = stat_pool.tile([128, 1], f32, tag="nmr", bufs=2)
            nc.vector.tensor_scalar(nmr, m, rstd, -1.0, ALU.mult, ALU.mult)
            x_bf = xbf_pool.tile([128, D], bf16, tag="xbf")
            if apply_ln1g or apply_ln1b:
                xf = s_pool.tile([128, D], f32, tag="xf")
                nc.scalar.activation(xf, s_sb, AF.Identity, bias=nmr, scale=rstd)
                if apply_ln1g:
                    nc.vector.tensor_mul(xf, xf, g1_sb)
                if apply_ln1b:
                    nc.vector.tensor_add(xf, xf, be1_sb)
                nc.vector.tensor_copy(x_bf, xf)
            else:
                nc.scalar.activation(x_bf, s_sb, AF.Identity, bias=nmr,
                                     scale=rstd)
            xbf[p] = x_bf

        def emit_transposes(p):
            blk, j = divmod(p, 4)
            if j == 0:
                xT_blks[blk] = xT_pool.tile([128, 4, 512], bf16, tag="xT",
                                            name="xT_blk")
            xT_blk = xT_blks[blk]
            x_bf = xbf[p]
            ps_xt = xt_ps.tile([128, 512], bf16, tag="xt",
                               padded_shape=[128, 1024])
            for dt in range(4):
                nc.tensor.transpose(ps_xt[:, dt * 128:(dt + 1) * 128],
                                    x_bf[:, dt * 128:(dt + 1) * 128], ident_sb)
            nc.vector.tensor_copy(
                xT_blk[:, :, j * 128:(j + 1) * 128],
                ps_xt.rearrange("p (dt q) -> p dt q", dt=4))

        def emit_ffn_h(blk):
            xT_blk = xT_blks.pop(blk)
            h_sb = h_pool.tile([128, 16, 512], bf16, tag="h")
            for ft in range(16):
                ps_h = big_ps.tile([128, 512], f32, tag="big")
                for kt in range(4):
                    nc.tensor.matmul(
                        ps_h,
                        lhsT=w1_sb[:, kt, ft * 128:(ft + 1) * 128],
                        rhs=xT_blk[:, kt, :],
                        start=(kt == 0), stop=(kt == 3),
                    )
                if ft % 2 == 0:
                    nc.scalar.activation(h_sb[:, ft, :], ps_h, AF.Relu,
                                         bias=b1T_sb[:, ft:ft + 1])
                else:
                    nc.vector.tensor_scalar(h_sb[:, ft, :], ps_h,
                                            b1T_sb[:, ft:ft + 1], 0.0,
                                            ALU.add, ALU.max)
            h_blks[blk] = h_sb

        def emit_ffn_y(blk):
            h_sb = h_blks.pop(blk)
            ln2[blk] = []
            for j in range(4):
                p = blk * 4 + j
                ps_y = big_ps.tile([128, 512], f32, tag="big")
                for ft in range(16):
                    nc.tensor.matmul(
                        ps_y,
                        lhsT=h_sb[:, ft, j * 128:(j + 1) * 128],
                        rhs=w2_sb[:, ft, :],
                        start=(ft == 0), stop=(ft == 15 and not apply_b2),
                    )
                if apply_b2:
                    nc.tensor.matmul(ps_y, lhsT=onerow2_sb, rhs=b2row_sb,
                                     start=False, stop=True)
                s2 = s2_pool.tile([128, D], f32, tag="s2")
                s2sum = stat_pool.tile([128, 1], f32, tag="s2sum")
                nc.vector.scalar_tensor_tensor(
                    s2, ps_y, 1.0, xbf.pop(p), ALU.mult, ALU.add,
                    accum_out=s2sum)
                junk = junk_pool.tile([128, D], bf16, tag="junk")
                sumsq2 = stat_pool.tile([128, 1], f32, tag="sumsq2")
                nc.scalar.activation(junk, s2, AF.Square, accum_out=sumsq2)
                m2 = stat_pool.tile([128, 1], f32, tag="m2")
                nc.vector.tensor_scalar_mul(m2, s2sum, INV_D)
                msq2 = stat_pool.tile([128, 1], f32, tag="msq2")
                nc.vector.tensor_scalar_mul(msq2, m2, m2)
                var2 = stat_pool.tile([128, 1], f32, tag="var2")
                nc.vector.scalar_tensor_tensor(var2, sumsq2, INV_D, msq2,
                                               ALU.mult, ALU.subtract)
                std2 = stat_pool.tile([128, 1], f32, tag="std2")
                nc.scalar.activation(std2, var2, AF.Sqrt, bias=eps_sb)
                rstd2 = stat_pool.tile([128, 1], f32, tag="rstd2")
                nc.vector.reciprocal(rstd2, std2)
                nmr2 = stat_pool.tile([128, 1], f32, tag="nmr2")
                nc.vector.tensor_scalar(nmr2, m2, rstd2, -1.0,
                                        ALU.mult, ALU.mult)
                ln2[blk].append((s2, rstd2, nmr2))

        def emit_ln2_tail(blk):
            for j, (s2, rstd2, nmr2) in enumerate(ln2.pop(blk)):
                p = blk * 4 + j
                o_sb = o_pool.tile([128, D], f32, tag="o")
                nc.scalar.activation(o_sb, s2, AF.Identity, bias=nmr2,
                                     scale=rstd2)
                if apply_ln2g:
                    nc.vector.tensor_mul(o_sb, o_sb, g2_sb)
                if apply_ln2b:
                    nc.vector.tensor_add(o_sb, o_sb, be2_sb)
                nc.sync.dma_start(out_d[p * 128:(p + 1) * 128, :], o_sb)

        # ---- phase 2 pipeline ----
        for s in range(NPAIR + 5):
            if s < NPAIR:
                emit_src_prefetch(s)
            ps_av = None
            if 1 <= s <= NPAIR:
                ps_av = emit_av_mms(s - 1)
            if s < NPAIR:
                emit_sim(s)
            if ps_av is not None:
                emit_pairchain(s - 1, ps_av)
            if 2 <= s < NPAIR + 2:
                emit_transposes(s - 2)
            if s >= 6 and (s - 6) % 4 == 0 and (s - 6) // 4 < NBLK:
                emit_ffn_h((s - 6) // 4)
            if s >= 7 and (s - 7) % 4 == 0 and (s - 7) // 4 < NBLK:
                emit_ffn_y((s - 7) // 4)
            if s >= 8 and (s - 8) % 4 == 0 and (s - 8) // 4 < NBLK:
                emit_ln2_tail((s - 8) // 4)

    nc.compile()
    return nc


def _get_program(key):
    if key not in _cache:
        _cache[key] = _build(*key)
    return _cache[key]


last_exec_ns = None


def _install_ntff_hook():
    """NTFF profiling hook for axon (normally installed via antenv.axon_hooks)."""
    import sys, types
    if 'antenv.axon_hooks' in sys.modules:
        return
    mod = types.ModuleType('antenv.axon_hooks')
    _h = [None]
    mod.set_axon_ntff_profile_hook = lambda h: _h.__setitem__(0, h)
    mod.get_axon_ntff_profile_hook = lambda: _h[0]
    sys.modules['antenv.axon_hooks'] = mod
    import antenv
    antenv.axon_hooks = mod
    try:
        from trn_agent_boot.trn_boot import _ntff_profile_via_ctypes
        mod.set_axon_ntff_profile_hook(
            _ntff_profile_via_ctypes('/opt/axon/libaxon_pjrt.so'))
    except Exception:
        pass


def kernel(src, mask, Wq, bq, Wk, bk, Wv, bv, ln1_g, ln1_b,
           W1, b1, W2, b2, ln2_g, ln2_b):
    global last_exec_ns
    src = np.asarray(src, np.float32)
    if not bool(np.asarray(mask).all()):
        raise NotImplementedError("only all-true mask supported")

    key = (bool(np.any(bv)), bool(np.any(b2)),
           not bool(np.all(ln1_g == 1)), bool(np.any(ln1_b)),
           not bool(np.all(ln2_g == 1)), bool(np.any(ln2_b)))
    nc = _get_program(key)
    apply_bv, apply_b2, a_g1, a_b1, a_g2, a_b2 = key

    qi = np.arange(128)
    wA = np.where(qi >= 64, NEG, 0.0).astype(_BF16).reshape(1, 128)
    wB = np.where(qi < 64, NEG, 0.0).astype(_BF16).reshape(1, 128)
    wfull = np.full((1, 128), NEG, _BF16)
    uA = (qi < 64).astype(_BF16).reshape(1, 128)
    uB = (qi >= 64).astype(_BF16).reshape(1, 128)

    shared = {
        "wq": Wq.astype(_BF16), "wk": Wk.astype(_BF16), "wv": Wv.astype(_BF16),
        "bqT": np.asarray(bq, np.float32).reshape(4, 128).T.copy(),
        "bkT": np.asarray(bk, np.float32).reshape(4, 128).T.copy(),
        "w1": W1.astype(_BF16),
        "b1T": np.asarray(b1, np.float32).reshape(16, 128).T.copy(),
        "w2": W2.astype(_BF16),
        "ident": np.eye(128, dtype=_BF16),
        "uA": uA, "uB": uB, "wA": wA, "wB": wB,
        "onerow": np.ones((1, 128), _BF16),
    }
    if apply_bv:
        shared["bvrow"] = np.asarray(bv, np.float32).reshape(1, D).astype(_BF16)
    if apply_b2:
        shared["b2row"] = np.asarray(b2, np.float32).reshape(1, D).astype(_BF16)
    if a_g1:
        shared["g1"] = np.tile(np.asarray(ln1_g, np.float32).reshape(1, D), (128, 1))
    if a_b1:
        shared["be1"] = np.tile(np.asarray(ln1_b, np.float32).reshape(1, D), (128, 1))
    if a_g2:
        shared["g2"] = np.tile(np.asarray(ln2_g, np.float32).reshape(1, D), (128, 1))
    if a_b2:
        shared["be2"] = np.tile(np.asarray(ln2_b, np.float32).reshape(1, D), (128, 1))

    in_maps = []
    for c in range(8):
        b, h = divmod(c, 2)
        start = h * T - H
        ext = np.zeros((TEXT, D), np.float32)
        lo, hi = max(start, 0), min(start + TEXT, N)
        ext[lo - start: hi - start] = src[b, lo:hi]
        m = dict(shared)
        m["srcT"] = np.ascontiguousarray(ext.T).astype(_BF16)
        m["srcbf"] = np.ascontiguousarray(src[b, h * T:(h + 1) * T]).astype(_BF16)
        m["wA0"] = wfull if h == 0 else wA
        m["wB31"] = wfull if h == 1 else wB
        in_maps.append(m)

    from concourse.bass_utils import run_bass_kernel_spmd
    trace = bool(os.environ.get("KERNEL_TRACE"))
    if trace:
        _install_ntff_hook()
    res = run_bass_kernel_spmd(nc, in_maps, core_ids=list(range(8)), trace=trace)
    if trace:
        last_exec_ns = res.exec_time_ns

    out = np.empty((B, N, D), np.float32)
    for c in range(8):
        b, h = divmod(c, 2)
        out[b, h * T:(h + 1) * T] = res.results[c]["out"]
    return out


# revision 17
# speedup vs baseline: 1.1970x; 1.0308x over previous
"""LocalTransformerEncoderLayer on 8 trn2 NeuronCores.

Sharding: core c = 2*b + h handles batch b, sequence half h (4096 tokens,
plus a 64-token halo on each side for the local-attention window).
Everything is done on-device per core; no collectives needed.

v2 schedule: per-pair software pipeline so the PE never stalls on the
DVE FIFO (the v1 per-block LN chains caused ~9us PE idle per block and
HAM re-throttling).

Layout plan (per core):
  srcT  [512, 4224] bf16  d-major haloed chunk (host-transposed)  -> QKV rhs/lhsT
  qT,kT [128,4,4224] bf16 d-major in SBUF (PE: W.T @ srcT)
  v     [128,33,512] bf16 token-major in SBUF (PE: srcT.T @ Wv)
  per q-pair p (128 query tokens, 256 keys = ext tiles p,p+1):
    step p:   simT [256 keys, 128 q] psum (+rank-1 corner masks); exp (ACT);
              den = expT.T @ ones (PE, same psum bank)
    step p+1: av = expT.T @ v (PE); s = av*recip + src (DVE stt, accum=sum);
              sumsq via ACT square(accum); m/var (gpsimd); sqrt (ACT);
              rstd (DVE); x_bf = (s-m)*rstd (gpsimd, bf16)
    step p+2: 4 PE transposes of x_bf into one psum bank; 1 strided copy out
  FFN per 512-token block b (steps 4b+6 / 4b+7):
    h[f,tok] = relu(W1.T @ xT) (PE + ACT/DVE split relu)
    y[tok,d] = h.T @ W2 (PE); s2 = y + x (DVE stt, accum); LN2 stats like LN1
  step 4b+8: o = (s2-m2)*rstd2 (gpsimd); DMA out fp32.
"""
import os
import numpy as np
import ml_dtypes

_BF16 = ml_dtypes.bfloat16

B, N, D, F, W = 4, 8192, 512, 2048, 64
T = N // 2            # own tokens per core = 4096
H = 64                # halo
TEXT = T + 2 * H      # 4224
NPAIR = T // 128      # 32 q-pairs per core
NBLK = T // 512       # 8 blocks
NEG = -1e10
SCALE = float(D) ** -0.5
INV_D = 1.0 / float(D)

_cache = {}


def _build(apply_bv, apply_b2, apply_ln1g, apply_ln1b, apply_ln2g, apply_ln2b,
           apply_b1):
    import concourse.bacc as bacc
    import concourse.tile as tile
    from concourse import mybir
    import concourse.bass as bass

    f32 = mybir.dt.float32
    bf16 = mybir.dt.bfloat16
    AF = mybir.ActivationFunctionType
    ALU = mybir.AluOpType

    nc = bacc.Bacc("TRN2", target_bir_lowering=False, debug=False)

    # ---- DRAM I/O ----
    srcT_d = nc.dram_tensor("srcT", [D, TEXT], bf16, kind="ExternalInput").ap()
    srcbf_d = nc.dram_tensor("srcbf", [T, D], bf16, kind="ExternalInput").ap()
    wq_d = nc.dram_tensor("wq", [D, D], bf16, kind="ExternalInput").ap()
    wk_d = nc.dram_tensor("wk", [D, D], bf16, kind="ExternalInput").ap()
    wv_d = nc.dram_tensor("wv", [D, D], bf16, kind="ExternalInput").ap()
    bqT_d = nc.dram_tensor("bqT", [128, 4], f32, kind="ExternalInput").ap()
    bkT_d = nc.dram_tensor("bkT", [128, 4], f32, kind="ExternalInput").ap()
    w1_d = nc.dram_tensor("w1", [D, F], bf16, kind="ExternalInput").ap()
    b1T_d = nc.dram_tensor("b1T", [128, 16], f32, kind="ExternalInput").ap()
    w2_d = nc.dram_tensor("w2", [F, D], bf16, kind="ExternalInput").ap()
    ident_d = nc.dram_tensor("ident", [128, 128], bf16, kind="ExternalInput").ap()
    uA_d = nc.dram_tensor("uA", [1, 128], bf16, kind="ExternalInput").ap()
    uB_d = nc.dram_tensor("uB", [1, 128], bf16, kind="ExternalInput").ap()
    wA_d = nc.dram_tensor("wA", [1, 128], bf16, kind="ExternalInput").ap()
    wB_d = nc.dram_tensor("wB", [1, 128], bf16, kind="ExternalInput").ap()
    wA0_d = nc.dram_tensor("wA0", [1, 128], bf16, kind="ExternalInput").ap()
    wB31_d = nc.dram_tensor("wB31", [1, 128], bf16, kind="ExternalInput").ap()
    onerow_d = nc.dram_tensor("onerow", [1, 128], bf16, kind="ExternalInput").ap()
    if apply_bv:
        bvrow_d = nc.dram_tensor("bvrow", [1, D], bf16, kind="ExternalInput").ap()
    if apply_b2:
        b2row_d = nc.dram_tensor("b2row", [1, D], bf16, kind="ExternalInput").ap()
    if apply_ln1g:
        g1_d = nc.dram_tensor("g1", [128, D], f32, kind="ExternalInput").ap()
    if apply_ln1b:
        be1_d = nc.dram_tensor("be1", [128, D], f32, kind="ExternalInput").ap()
    if apply_ln2g:
        g2_d = nc.dram_tensor("g2", [128, D], f32, kind="ExternalInput").ap()
    if apply_ln2b:
        be2_d = nc.dram_tensor("be2", [128, D], f32, kind="ExternalInput").ap()
    out_d = nc.dram_tensor("out", [T, D], f32, kind="ExternalOutput").ap()

    from contextlib import ExitStack
    with tile.TileContext(nc) as tc, ExitStack() as ctx:
        # ---- pools ----
        consts = ctx.enter_context(tc.tile_pool(name="consts", bufs=1))
        kv = ctx.enter_context(tc.tile_pool(name="kv", bufs=1))
        big_ps = ctx.enter_context(tc.tile_pool(name="big_ps", bufs=2, space="PSUM"))
        av_ps = ctx.enter_context(tc.tile_pool(name="av_ps", bufs=2, space="PSUM"))
        sim_ps = ctx.enter_context(tc.tile_pool(name="sim_ps", bufs=2, space="PSUM"))
        xt_ps = ctx.enter_context(tc.tile_pool(name="xt_ps", bufs=2, space="PSUM"))

        srcs = ctx.enter_context(tc.tile_pool(name="srcs", bufs=2))
        io_pool = ctx.enter_context(tc.tile_pool(name="io_pool", bufs=2))
        exp_pool = ctx.enter_context(tc.tile_pool(name="exp_pool", bufs=2))
        s_pool = ctx.enter_context(tc.tile_pool(name="s_pool", bufs=2))
        xbf_pool = ctx.enter_context(tc.tile_pool(name="xbf_pool", bufs=7))
        xT_pool = ctx.enter_context(tc.tile_pool(name="xT_pool", bufs=2))
        h_pool = ctx.enter_context(tc.tile_pool(name="h_pool", bufs=1))
        s2_pool = ctx.enter_context(tc.tile_pool(name="s2_pool", bufs=4))
        o_pool = ctx.enter_context(tc.tile_pool(name="o_pool", bufs=3))
        stat_pool = ctx.enter_context(tc.tile_pool(name="stat_pool", bufs=4))

        # ---- load-bearing DMAs first: srcT chunk 0, wq, wk ----
        srcT_r = srcT_d.rearrange("(dt p) t -> p dt t", p=128)
        blocks = [(i * 512, 512) for i in range(TEXT // 512)] + [(4096, 128)]
        srcT_tiles = {}
        off0, tw0 = blocks[0]
        srcT0 = srcs.tile([128, 4, 512], bf16, tag="srcT")
        nc.sync.dma_start(srcT0[:, :, :tw0], srcT_r[:, :, off0:off0 + tw0])
        srcT_tiles[0] = srcT0
        wq_sb = consts.tile([128, 4, D], bf16, tag="wq")
        nc.sync.dma_start(wq_sb, wq_d.rearrange("(kt p) m -> p kt m", p=128))
        wk_sb = consts.tile([128, 4, D], bf16, tag="wk")
        nc.sync.dma_start(wk_sb, wk_d.rearrange("(kt p) m -> p kt m", p=128))
        bqT_sb = consts.tile([128, 4], f32, tag="bqT")
        nc.sync.dma_start(bqT_sb, bqT_d)
        bkT_sb = consts.tile([128, 4], f32, tag="bkT")
        nc.sync.dma_start(bkT_sb, bkT_d)
        wv_sb = consts.tile([128, 4, D], bf16, tag="wv")
        nc.sync.dma_start(wv_sb, wv_d.rearrange("(kt p) m -> p kt m", p=128))

        w1_sb = consts.tile([128, 4, F], bf16, tag="w1")
        w2_sb = consts.tile([128, 16, D], bf16, tag="w2")
        b1T_sb = consts.tile([128, 16], f32, tag="b1T")
        nc.sync.dma_start(b1T_sb, b1T_d)
        ident_sb = consts.tile([128, 128], bf16, tag="ident")
        nc.sync.dma_start(ident_sb, ident_d)
        uA_sb = consts.tile([1, 128], bf16, tag="uA")
        nc.sync.dma_start(uA_sb, uA_d)
        uB_sb = consts.tile([1, 128], bf16, tag="uB")
        nc.sync.dma_start(uB_sb, uB_d)
        wA_sb = consts.tile([1, 128], bf16, tag="wA")
        nc.sync.dma_start(wA_sb, wA_d)
        wB_sb = consts.tile([1, 128], bf16, tag="wB")
        nc.sync.dma_start(wB_sb, wB_d)
        wA0_sb = consts.tile([1, 128], bf16, tag="wA0")
        nc.sync.dma_start(wA0_sb, wA0_d)
        wB31_sb = consts.tile([1, 128], bf16, tag="wB31")
        nc.sync.dma_start(wB31_sb, wB31_d)
        ones_sb = consts.tile([128, 1], bf16, tag="ones")
        nc.vector.memset(ones_sb, 1.0)
        u32 = mybir.dt.uint32
        magic_sb = consts.tile([128, 1], u32, tag="magic")
        nc.vector.memset(magic_sb, 0x5F3759DF)
        # ReLU-homogeneity fast path: feed FFN the centered-but-unnormalized
        # residual and fold rstd into the y output. Valid only when LN1 is
        # affine-free and the FFN biases are zero.
        fast = not (apply_ln1g or apply_ln1b or apply_b1 or apply_b2)
        if apply_bv:
            onerow_sb = consts.tile([1, 128], bf16, tag="onerow")
            nc.sync.dma_start(onerow_sb, onerow_d)
            bvrow_sb = consts.tile([1, D], bf16, tag="bvrow")
            nc.sync.dma_start(bvrow_sb, bvrow_d)
        if apply_b2:
            onerow2_sb = consts.tile([1, 128], bf16, tag="onerow2")
            nc.sync.dma_start(onerow2_sb, onerow_d)
            b2row_sb = consts.tile([1, D], bf16, tag="b2row")
            nc.sync.dma_start(b2row_sb, b2row_d)
        if apply_ln1g:
            g1_sb = consts.tile([128, D], f32, tag="g1")
            nc.sync.dma_start(g1_sb, g1_d)
        if apply_ln1b:
            be1_sb = consts.tile([128, D], f32, tag="be1")
            nc.sync.dma_start(be1_sb, be1_d)
        if apply_ln2g:
            g2_sb = consts.tile([128, D], f32, tag="g2")
            nc.sync.dma_start(g2_sb, g2_d)
        if apply_ln2b:
            be2_sb = consts.tile([128, D], f32, tag="be2")
            nc.sync.dma_start(be2_sb, be2_d)

        # persistent activations (qT only covers own tokens, no halo)
        qT_sb = kv.tile([128, 4, T], bf16, tag="qT")
        kT_sb = kv.tile([128, 4, TEXT], bf16, tag="kT")
        v_sb = kv.tile([128, 33, D], bf16, tag="v")

        # ---- phase 1: QKV over ext grid ----
        for bi, (off, tw) in enumerate(blocks):
            if bi + 1 < len(blocks):
                noff, ntw = blocks[bi + 1]
                srcT_n = srcs.tile([128, 4, 512], bf16, tag="srcT")
                nc.sync.dma_start(srcT_n[:, :, :ntw], srcT_r[:, :, noff:noff + ntw])
                srcT_tiles[bi + 1] = srcT_n
            srcT_sb = srcT_tiles.pop(bi)
            # q range clipped to own tokens [H, H+T) in ext coords
            qlo, qhi = max(off, H), min(off + tw, H + T)
            # qT, kT (d-major)
            for w_sb, b_sb, dst, lo, hi, doff in (
                (wq_sb, bqT_sb, qT_sb, qlo, qhi, -H),
                (wk_sb, bkT_sb, kT_sb, off, off + tw, 0),
            ):
                if lo >= hi:
                    continue
                for dq in range(4):
                    ps = big_ps.tile([128, 512], f32, tag="big")
                    for kt in range(4):
                        nc.tensor.matmul(
                            ps[:, :tw],
                            lhsT=w_sb[:, kt, dq * 128:(dq + 1) * 128],
                            rhs=srcT_sb[:, kt, :tw],
                            start=(kt == 0), stop=(kt == 3),
                        )
                    nc.scalar.activation(
                        dst[:, dq, lo + doff:hi + doff],
                        ps[:, lo - off:hi - off],
                        AF.Identity, bias=b_sb[:, dq:dq + 1],
                    )
            # v (token-major) into SBUF, per 128-token tile
            for s in range(tw // 128):
                ti = (off + s * 128) // 128
                ps = big_ps.tile([128, 512], f32, tag="big")
                for kt in range(4):
                    nc.tensor.matmul(
                        ps,
                        lhsT=srcT_sb[:, kt, s * 128:s * 128 + 128],
                        rhs=wv_sb[:, kt, :],
                        start=(kt == 0), stop=(kt == 3 and not apply_bv),
                    )
                if apply_bv:
                    nc.tensor.matmul(ps, lhsT=onerow_sb, rhs=bvrow_sb,
                                     start=False, stop=True)
                nc.vector.tensor_copy(v_sb[:, ti, :], ps)

        # FFN weights needed ~150us in; emit their DMAs after phase 1
        nc.sync.dma_start(w1_sb, w1_d.rearrange("(kt p) m -> p kt m", p=128))
        nc.sync.dma_start(w2_sb, w2_d.rearrange("(ft p) m -> p ft m", p=128))

        # ---- phase 2 state ----
        simden = {}       # p -> psum tile: [:, 0:256] simT, [:, 256:257] den
        expT_t = {}
        src_t = {}        # residual src tiles (bf16)
        s_tiles = {}      # p -> (s_f32, m, rstd)
        xbf = {}          # p -> bf16 LN1 output
        xT_blks = {}
        h_blks = {}
        ln2 = {}          # b -> list of (s2, m2, rstd2)

        def emit_src_prefetch(p):
            t = io_pool.tile([128, D], bf16, tag="srct")
            nc.sync.dma_start(t, srcbf_d[p * 128:(p + 1) * 128, :])
            src_t[p] = t

        def emit_sim(p):
            qoff = p * 128
            ps = sim_ps.tile([128, 512], f32, tag="sim")
            simden[p] = ps
            for half, (ktile, u_sb, w_vec) in enumerate((
                (p, uA_sb, wA0_sb if p == 0 else wA_sb),
                (p + 1, uB_sb, wB31_sb if p == NPAIR - 1 else wB_sb),
            )):
                reg = ps[:, half * 128:(half + 1) * 128]
                for kt in range(4):
                    nc.tensor.matmul(
                        reg,
                        lhsT=kT_sb[:, kt, ktile * 128:(ktile + 1) * 128],
                        rhs=qT_sb[:, kt, qoff:qoff + 128],
                        start=(kt == 0), stop=False,
                    )
                nc.tensor.matmul(reg, lhsT=u_sb, rhs=w_vec, start=False, stop=True)
            expT = exp_pool.tile([128, 256], bf16, tag="expT")
            nc.scalar.activation(expT, ps[:, 0:256], AF.Exp, scale=SCALE)
            expT_t[p] = expT

        def emit_av_mms(p):
            # den + av matmuls; all inputs were produced during step p, so
            # these never make the PE wait on another engine.
            expT = expT_t.pop(p)
            ps_sd = simden[p]
            nc.tensor.matmul(ps_sd[:, 256:257], lhsT=expT[:, 0:128], rhs=ones_sb,
                             start=True, stop=False)
            nc.tensor.matmul(ps_sd[:, 256:257], lhsT=expT[:, 128:256], rhs=ones_sb,
                             start=False, stop=True)
            ps_av = av_ps.tile([128, 512], f32, tag="av")
            nc.tensor.matmul(ps_av, lhsT=expT[:, 0:128], rhs=v_sb[:, p, :],
                             start=True, stop=False)
            nc.tensor.matmul(ps_av, lhsT=expT[:, 128:256], rhs=v_sb[:, p + 1, :],
                             start=False, stop=True)
            return ps_av

        def emit_rsqrt(var_ap, tag, final_bufs):
            """rstd = 1/sqrt(var + eps), entirely on DVE (no ACT table)."""
            ve = stat_pool.tile([128, 1], f32, tag=tag + "ve", bufs=2, name="ve")
            nc.vector.tensor_scalar_add(ve, var_ap, 1e-5)
            ti = stat_pool.tile([128, 1], u32, tag=tag + "ti", bufs=2, name="ti")
            nc.vector.tensor_scalar(ti, ve.bitcast(u32), 1, None,
                                    ALU.logical_shift_right)
            r = stat_pool.tile([128, 1], f32, tag=tag + "r0", bufs=2, name="r0")
            nc.vector.scalar_tensor_tensor(r.bitcast(u32), magic_sb, 0, ti,
                                           ALU.bypass, ALU.subtract)
            for it in range(2):
                last = it == 1
                a = stat_pool.tile([128, 1], f32, tag=tag + "a%d" % it,
                                   bufs=2, name="a")
                nc.vector.tensor_scalar(a, r, r, None, ALU.mult)
                b_ = stat_pool.tile([128, 1], f32, tag=tag + "b%d" % it,
                                    bufs=2, name="b_")
                nc.vector.tensor_scalar(b_, a, ve, -0.5, ALU.mult, ALU.mult)
                rn = stat_pool.tile([128, 1], f32, tag=tag + "r%d" % (it + 1),
                                    bufs=final_bufs if last else 2, name="rn")
                nc.vector.scalar_tensor_tensor(rn, b_, 1.5, r,
                                               ALU.add, ALU.mult)
                r = rn
            return r

        rstd1 = {}

        def emit_pairchain(p, ps_av):
            ps_sd = simden.pop(p)
            recip = stat_pool.tile([128, 1], f32, tag="recip", bufs=2)
            nc.vector.reciprocal(recip, ps_sd[:, 256:257])
            s_sb = s_pool.tile([128, D], f32, tag="s")
            ssum = stat_pool.tile([128, 1], f32, tag="ssum", bufs=2)
            nc.vector.scalar_tensor_tensor(
                s_sb, ps_av, recip, src_t.pop(p),
                ALU.mult, ALU.add, accum_out=ssum)
            if fast:
                # critical path: mean-center only; rstd applied at y-stage
                m = stat_pool.tile([128, 1], f32, tag="m", bufs=2)
                nc.vector.tensor_scalar_mul(m, ssum, INV_D)
                s_c = xbf_pool.tile([128, D], bf16, tag="xbf")
                nc.vector.tensor_scalar(s_c, s_sb, m, None, ALU.subtract)
                xbf[p] = s_c
                # deferred: variance of centered values -> rstd
                st6 = stat_pool.tile([128, 6], f32, tag="st6", bufs=2)
                nc.vector.bn_stats(st6, s_c)
                mv = stat_pool.tile([128, 2], f32, tag="mv", bufs=2)
                nc.vector.bn_aggr(mv, st6)
                rstd1[p] = emit_rsqrt(mv[:, 1:2], "q1", 8)
            else:
                st6 = stat_pool.tile([128, 6], f32, tag="st6", bufs=2)
                nc.vector.bn_stats(st6, s_sb)
                mv = stat_pool.tile([128, 2], f32, tag="mv", bufs=2)
                nc.vector.bn_aggr(mv, st6)
                rstd = emit_rsqrt(mv[:, 1:2], "q1", 2)
                nmr = stat_pool.tile([128, 1], f32, tag="nmr", bufs=2)
                nc.vector.tensor_scalar(nmr, mv[:, 0:1], rstd, -1.0,
                                        ALU.mult, ALU.mult)
                x_bf = xbf_pool.tile([128, D], bf16, tag="xbf")
                if apply_ln1g or apply_ln1b:
                    xf = s_pool.tile([128, D], f32, tag="xf")
                    nc.scalar.activation(xf, s_sb, AF.Identity, bias=nmr,
                                         scale=rstd)
                    if apply_ln1g:
                        nc.vector.tensor_mul(xf, xf, g1_sb)
                    if apply_ln1b:
                        nc.vector.tensor_add(xf, xf, be1_sb)
                    nc.vector.tensor_copy(x_bf, xf)
                else:
                    nc.scalar.activation(x_bf, s_sb, AF.Identity, bias=nmr,
                                         scale=rstd)
                xbf[p] = x_bf

        def emit_transposes(p):
            blk, j = divmod(p, 4)
            if j == 0:
                xT_blks[blk] = xT_pool.tile([128, 4, 512], bf16, tag="xT",
                                            name="xT_blk")
            xT_blk = xT_blks[blk]
            x_bf = xbf[p]
            ps_xt = xt_ps.tile([128, 512], bf16, tag="xt",
                               padded_shape=[128, 1024])
            for dt in range(4):
                nc.tensor.transpose(ps_xt[:, dt * 128:(dt + 1) * 128],
                                    x_bf[:, dt * 128:(dt + 1) * 128], ident_sb)
            nc.vector.tensor_copy(
                xT_blk[:, :, j * 128:(j + 1) * 128],
                ps_xt.rearrange("p (dt q) -> p dt q", dt=4))

        def emit_ffn_h(blk):
            xT_blk = xT_blks.pop(blk)
            h_sb = h_pool.tile([128, 16, 512], bf16, tag="h")
            for ft in range(16):
                ps_h = big_ps.tile([128, 512], f32, tag="big")
                for kt in range(4):
                    nc.tensor.matmul(
                        ps_h,
                        lhsT=w1_sb[:, kt, ft * 128:(ft + 1) * 128],
                        rhs=xT_blk[:, kt, :],
                        start=(kt == 0), stop=(kt == 3),
                    )
                if fast:
                    nc.scalar.activation(h_sb[:, ft, :], ps_h, AF.Relu)
                elif ft % 2 == 0:
                    nc.scalar.activation(h_sb[:, ft, :], ps_h, AF.Relu,
                                         bias=b1T_sb[:, ft:ft + 1])
                else:
                    nc.vector.tensor_scalar(h_sb[:, ft, :], ps_h,
                                            b1T_sb[:, ft:ft + 1], 0.0,
                                            ALU.add, ALU.max)
            h_blks[blk] = h_sb

        def emit_ffn_y(blk):
            h_sb = h_blks.pop(blk)
            ln2[blk] = []
            for j in range(4):
                p = blk * 4 + j
                ps_y = big_ps.tile([128, 512], f32, tag="big")
                for ft in range(16):
                    nc.tensor.matmul(
                        ps_y,
                        lhsT=h_sb[:, ft, j * 128:(j + 1) * 128],
                        rhs=w2_sb[:, ft, :],
                        start=(ft == 0), stop=(ft == 15 and not apply_b2),
                    )
                if apply_b2:
                    nc.tensor.matmul(ps_y, lhsT=onerow2_sb, rhs=b2row_sb,
                                     start=False, stop=True)
                s2 = s2_pool.tile([128, D], f32, tag="s2")
                if fast:
                    # s2 = rstd * (s_c + y_raw)  (relu homogeneity)
                    nc.vector.tensor_add(s2, ps_y, xbf.pop(p))
                    nc.vector.tensor_scalar_mul(s2, s2, rstd1.pop(p))
                else:
                    nc.vector.scalar_tensor_tensor(
                        s2, ps_y, 1.0, xbf.pop(p), ALU.mult, ALU.add)
                st6 = stat_pool.tile([128, 6], f32, tag="st6b", bufs=2)
                nc.vector.bn_stats(st6, s2)
                mv2 = stat_pool.tile([128, 2], f32, tag="mv2", bufs=2)
                nc.vector.bn_aggr(mv2, st6)
                rstd2 = emit_rsqrt(mv2[:, 1:2], "q2", 5)
                nmr2 = stat_pool.tile([128, 1], f32, tag="nmr2", bufs=5)
                nc.vector.tensor_scalar(nmr2, mv2[:, 0:1], rstd2, -1.0,
                                        ALU.mult, ALU.mult)
                ln2[blk].append((s2, rstd2, nmr2))

        def emit_ln2_tail(blk):
            for j, (s2, rstd2, nmr2) in enumerate(ln2.pop(blk)):
                p = blk * 4 + j
                o_sb = o_pool.tile([128, D], f32, tag="o")
                nc.scalar.activation(o_sb, s2, AF.Identity, bias=nmr2,
                                     scale=rstd2)
                if apply_ln2g:
                    nc.vector.tensor_mul(o_sb, o_sb, g2_sb)
                if apply_ln2b:
                    nc.vector.tensor_add(o_sb, o_sb, be2_sb)
                nc.sync.dma_start(out_d[p * 128:(p + 1) * 128, :], o_sb)

        # ---- phase 2 pipeline ----
        for s in range(NPAIR + 5):
            if s < NPAIR:
                emit_src_prefetch(s)
            ps_av = None
            if 1 <= s <= NPAIR:
                ps_av = emit_av_mms(s - 1)
            if s < NPAIR:
                emit_sim(s)
            if ps_av is not None:
                emit_pairchain(s - 1, ps_av)
            if 2 <= s < NPAIR + 2:
                emit_transposes(s - 2)
            if s >= 6 and (s - 6) % 4 == 0 and (s - 6) // 4 < NBLK:
                emit_ffn_h((s - 6) // 4)
            if s >= 7 and (s - 7) % 4 == 0 and (s - 7) // 4 < NBLK:
                emit_ffn_y((s - 7) // 4)
            if s >= 8 and (s - 8) % 4 == 0 and (s - 8) // 4 < NBLK:
                emit_ln2_tail((s - 8) // 4)

    nc.compile()
    return nc


def _get_program(key):
    if key not in _cache:
        _cache[key] = _build(*key)
    return _cache[key]


last_exec_ns = None


def _install_ntff_hook():
    """NTFF profiling hook for axon (normally installed via antenv.axon_hooks)."""
    import sys, types
    if 'antenv.axon_hooks' in sys.modules:
        return
    mod = types.ModuleType('antenv.axon_hooks')
    _h = [None]
    mod.set_axon_ntff_profile_hook = lambda h: _h.__setitem__(0, h)
    mod.get_axon_ntff_profile_hook = lambda: _h[0]
    sys.modules['antenv.axon_hooks'] = mod
    import antenv
    antenv.axon_hooks = mod
    try:
        from trn_agent_boot.trn_boot import _ntff_profile_via_ctypes
        mod.set_axon_ntff_profile_hook(
            _ntff_profile_via_ctypes('/opt/axon/libaxon_pjrt.so'))
    except Exception:
        pass


def kernel(src, mask, Wq, bq, Wk, bk, Wv, bv, ln1_g, ln1_b,
           W1, b1, W2, b2, ln2_g, ln2_b):
    global last_exec_ns
    src = np.asarray(src, np.float32)
    if not bool(np.asarray(mask).all()):
        raise NotImplementedError("only all-true mask supported")

    key = (bool(np.any(bv)), bool(np.any(b2)),
           not bool(np.all(ln1_g == 1)), bool(np.any(ln1_b)),
           not bool(np.all(ln2_g == 1)), bool(np.any(ln2_b)),
           bool(np.any(b1)))
    nc = _get_program(key)
    apply_bv, apply_b2, a_g1, a_b1, a_g2, a_b2, _ab1 = key

    qi = np.arange(128)
    wA = np.where(qi >= 64, NEG, 0.0).astype(_BF16).reshape(1, 128)
    wB = np.where(qi < 64, NEG, 0.0).astype(_BF16).reshape(1, 128)
    wfull = np.full((1, 128), NEG, _BF16)
    uA = (qi < 64).astype(_BF16).reshape(1, 128)
    uB = (qi >= 64).astype(_BF16).reshape(1, 128)

    shared = {
        "wq": Wq.astype(_BF16), "wk": Wk.astype(_BF16), "wv": Wv.astype(_BF16),
        "bqT": np.asarray(bq, np.float32).reshape(4, 128).T.copy(),
        "bkT": np.asarray(bk, np.float32).reshape(4, 128).T.copy(),
        "w1": W1.astype(_BF16),
        "b1T": np.asarray(b1, np.float32).reshape(16, 128).T.copy(),
        "w2": W2.astype(_BF16),
        "ident": np.eye(128, dtype=_BF16),
        "uA": uA, "uB": uB, "wA": wA, "wB": wB,
        "onerow": np.ones((1, 128), _BF16),
    }
    if apply_bv:
        shared["bvrow"] = np.asarray(bv, np.float32).reshape(1, D).astype(_BF16)
    if apply_b2:
        shared["b2row"] = np.asarray(b2, np.float32).reshape(1, D).astype(_BF16)
    if a_g1:
        shared["g1"] = np.tile(np.asarray(ln1_g, np.float32).reshape(1, D), (128, 1))
    if a_b1:
        shared["be1"] = np.tile(np.asarray(ln1_b, np.float32).reshape(1, D), (128, 1))
    if a_g2:
        shared["g2"] = np.tile(np.asarray(ln2_g, np.float32).reshape(1, D), (128, 1))
    if a_b2:
        shared["be2"] = np.tile(np.asarray(ln2_b, np.float32).reshape(1, D), (128, 1))

    in_maps = []
    for c in range(8):
        b, h = divmod(c, 2)
        start = h * T - H
        ext = np.zeros((TEXT, D), np.float32)
        lo, hi = max(start, 0), min(start + TEXT, N)
        ext[lo - start: hi - start] = src[b, lo:hi]
        m = dict(shared)
        m["srcT"] = np.ascontiguousarray(ext.T).astype(_BF16)
        m["srcbf"] = np.ascontiguousarray(src[b, h * T:(h + 1) * T]).astype(_BF16)
        m["wA0"] = wfull if h == 0 else wA
        m["wB31"] = wfull if h == 1 else wB
        in_maps.append(m)

    from concourse.bass_utils import run_bass_kernel_spmd
    trace = bool(os.environ.get("KERNEL_TRACE"))
    if trace:
        _install_ntff_hook()
    res = run_bass_kernel_spmd(nc, in_maps, core_ids=list(range(8)), trace=trace)
    if trace:
        last_exec_ns = res.exec_time_ns

    out = np.empty((B, N, D), np.float32)
    for c in range(8):
        b, h = divmod(c, 2)
        out[b, h * T:(h + 1) * T] = res.results[c]["out"]
    return out
